# revision 2
# baseline (speedup 1.0000x reference)
"""ACN2d multi-branch attentive normalization on 8 TRN2 NeuronCores.

Sharding: data-parallel over batch B (8 samples -> 8 cores), no collectives.
Per core, a two-phase Bass/Tile kernel:
  phase 1 (point-major): x is cast to fp16 (gpsimd) and transposed to
    point-major tiles by the XBAR DMA transpose (off the PE); att^T comes
    from per-tile PE matmuls; softmax over G=8 groups is a free-dim reduce
    on DVE; weighted moments P,Q accumulate on PE with a combined
    [x^T | x^2^T] moving operand.
  phase 2 (channel-major): a^T transposes back to group-major chunks on PE;
    A1/A2 are K=8 matmuls against inv_std^T / (mean inv_std)^T; DVE/ACT/
    gpsimd combine out = x*A1 - A2 in fp16 and results stream to HBM as
    fp16 (host casts to fp32).
"""

import numpy as np

from concourse.bass_utils import run_bass_kernel_spmd


from contextlib import ExitStack

import concourse.bass as bass
import concourse.bacc as bacc
import concourse.tile as tile
from concourse import mybir
from concourse.masks import make_identity

F32 = mybir.dt.float32
F16 = mybir.dt.float16
AF = mybir.ActivationFunctionType
OP = mybir.AluOpType
AX = mybir.AxisListType

EPS = 1e-3
A_NORM_EPS = 1e-8


def bcast_last(ap, n):
    return bass.AP(tensor=ap.tensor, offset=ap.offset, ap=list(ap.ap) + [[0, n]])


def build_nc(N=32768):
    C, G = 128, 8
    TP = 128
    GRP = 2048
    JJ = GRP // TP                # tiles per group = 16
    ntiles = N // TP
    ngrp = N // GRP
    assert N % GRP == 0

    nc = bacc.Bacc("TRN2", target_bir_lowering=False, debug=False)
    x_ext = nc.declare_dram_parameter("x", [C, N], F32, isOutput=False).ap()
    w_ext = nc.declare_dram_parameter("conv_w", [G, C], F32, isOutput=False).ap()
    b_ext = nc.declare_dram_parameter("conv_b", [1, G], F32, isOutput=False).ap()
    out_ext = nc.declare_dram_parameter("out", [C, N], F16, isOutput=True).ap()

    with tile.TileContext(nc) as tc, ExitStack() as ctx:
        consts = ctx.enter_context(tc.tile_pool(name="consts", bufs=1))
        resident = ctx.enter_context(tc.tile_pool(name="resident", bufs=1))
        stats = ctx.enter_context(tc.tile_pool(name="stats", bufs=1))
        xstage = ctx.enter_context(tc.tile_pool(name="xstage", bufs=2))
        xt2stage = ctx.enter_context(tc.tile_pool(name="xt2stage", bufs=2))
        estage = ctx.enter_context(tc.tile_pool(name="estage", bufs=2))
        zstage = ctx.enter_context(tc.tile_pool(name="zstage", bufs=2))
        ag_sbp = ctx.enter_context(tc.tile_pool(name="ag_sb", bufs=2))
        a1s_pool = ctx.enter_context(tc.tile_pool(name="a1s_sb", bufs=2))
        tm_pool = ctx.enter_context(tc.tile_pool(name="tm_sb", bufs=2))
        opool = ctx.enter_context(tc.tile_pool(name="ostage", bufs=2))

        # ---- constants ----
        ident = consts.tile([128, 128], F16)
        make_identity(nc, ident)
        ident8 = consts.tile([G, G], F32)
        make_identity(nc, ident8)
        onesf32 = consts.tile([128, 1], F32)
        nc.vector.memset(onesf32, 1.0)
        eps_t = consts.tile([G, 1], F32)
        nc.vector.memset(eps_t, EPS)
        w_sb = consts.tile([G, C], F32)
        nc.sync.dma_start(w_sb[:], w_ext)
        b_row = consts.tile([1, G], F32)
        nc.sync.dma_start(b_row[:], b_ext)
        ones_col = consts.tile([1, 128], F16)
        nc.gpsimd.memset(ones_col, 1.0)
        b_rep = consts.tile([1, JJ, G], F16)
        nc.gpsimd.tensor_copy(
            b_rep[:],
            bass.AP(tensor=b_row[:].tensor, offset=b_row[:].offset,
                    ap=[b_row[:].ap[0], [0, JJ], b_row[:].ap[1]]))
        w_g = consts.tile([G, C], F32)
        nc.gpsimd.tensor_copy(w_g[:], w_sb[:])

        with tc.tile_pool(name="ph0psum", bufs=1, space="PSUM") as ph0psum:
            wT_ps = ph0psum.tile([C, G], F32)
            nc.tensor.transpose(wT_ps[:], w_g[:], ident8[:])
            wT = consts.tile([C, G], F16)
            nc.scalar.copy(wT[:], wT_ps[:])

        # ---- residents ----
        xc = resident.tile([C, N], F16)
        a_t = resident.tile([128, N // 16], F16)

        # ================= phase 1 =================
        with tc.tile_pool(name="att_ps", bufs=2, space="PSUM") as att_pool, \
             tc.tile_pool(name="pq_ps", bufs=1, space="PSUM") as pq_pool:
            pq = pq_pool.tile([G, 2 * C], F32)   # [P^T | Q^T] interleaved

            prev = None  # previous group's xt2 tile for deferred P/Q
            for g in range(ngrp):
                n0 = g * GRP
                xs = xstage.tile([C, GRP], F32)
                nc.sync.dma_start(xs[:], x_ext[:, n0:n0 + GRP])
                xcs = xc[:, n0:n0 + GRP]
                nc.gpsimd.tensor_copy(xcs, xs[:])     # cast f32 -> f16

                # XBAR DMA transpose: xt2[:, j, 0:128] = (xc tile j)^T
                xt2 = xt2stage.tile([128, JJ, 2 * TP], F16)
                nc.scalar.dma_start_transpose(xt2[:, :, 0:TP], xcs)

                att = att_pool.tile([128, JJ, G], F32)
                for j in range(JJ):
                    xcj = xc[:, n0 + j * TP:n0 + (j + 1) * TP]
                    nc.tensor.matmul(
                        att[:, j, :], lhsT=xcj, rhs=wT[:],
                        start=(j == 0), stop=False, skip_group_check=True)
                nc.tensor.matmul(
                    att[:].rearrange("p j g -> p (j g)"),
                    lhsT=ones_col[:], rhs=b_rep[:].rearrange("p j g -> p (j g)"),
                    start=False, stop=True, skip_group_check=True)

                # deferred P/Q for the previous group keeps PE busy while this
                # group's transpose + x^2 land
                if prev is not None:
                    gg, xt2p = prev
                    for j in range(JJ):
                        t = gg * JJ + j
                        nc.tensor.matmul(
                            pq[:], lhsT=a_t[:, 8 * t:8 * (t + 1)],
                            rhs=xt2p[:, j, :],
                            start=(t == 0), stop=(t == ntiles - 1),
                            skip_group_check=True)

                e = estage.tile([128, JJ, G], F32)
                nc.scalar.activation(e[:], att[:], AF.Exp)
                z = zstage.tile([128, JJ], F32)
                nc.vector.tensor_reduce(z[:], e[:], axis=AX.X, op=OP.add)
                rz = zstage.tile([128, JJ], F32)
                nc.vector.reciprocal(rz[:], z[:])
                atG = a_t[:, g * (GRP // 16):(g + 1) * (GRP // 16)].rearrange(
                    "p (j g) -> p j g", g=G)
                nc.vector.tensor_tensor(atG, e[:], bcast_last(rz[:], G), op=OP.mult)

                # x^2 into the odd slots; alternate DVE / ACT for balance
                if g % 2 == 0:
                    nc.vector.tensor_tensor(
                        xt2[:, :, TP:2 * TP], xt2[:, :, 0:TP], xt2[:, :, 0:TP],
                        op=OP.mult)
                else:
                    nc.scalar.activation(
                        xt2[:, :, TP:2 * TP], xt2[:, :, 0:TP], AF.Square)
                prev = (g, xt2)

            gg, xt2p = prev
            for j in range(JJ):
                t = gg * JJ + j
                nc.tensor.matmul(
                    pq[:], lhsT=a_t[:, 8 * t:8 * (t + 1)], rhs=xt2p[:, j, :],
                    start=(t == 0), stop=(t == ntiles - 1), skip_group_check=True)

            # ================= phase 1.5: statistics =================
            partials = stats.tile([128, G], F32)
            nc.vector.tensor_reduce(
                partials[:], a_t[:].rearrange("p (t g) -> p g t", g=G),
                axis=AX.X, op=OP.add)
            with tc.tile_pool(name="s_ps", bufs=1, space="PSUM") as s_pool:
                s_ps = s_pool.tile([G, 1], F32)
                nc.tensor.matmul(s_ps[:], lhsT=partials[:], rhs=onesf32[:],
                                 start=True, stop=True)
                s_eps = stats.tile([G, 1], F32)
                nc.vector.tensor_scalar_add(s_eps[:], s_ps[:], A_NORM_EPS)
                sden = stats.tile([G, 1], F32)
                nc.vector.reciprocal(sden[:], s_eps[:])
                T = stats.tile([G, 1], F32)
                nc.vector.tensor_tensor(T[:], s_ps[:], sden[:], op=OP.mult)
                meanT = stats.tile([G, C], F32)
                nc.vector.tensor_scalar_mul(meanT[:], pq[:, 0:C], sden[:])
                m2T = stats.tile([G, C], F32)
                nc.vector.tensor_scalar_mul(m2T[:], pq[:, C:2 * C], sden[:])
            u = stats.tile([G, 1], F32)
            nc.vector.tensor_scalar(u[:], T[:], -1.0, 2.0, op0=OP.mult, op1=OP.add)
            meansq = stats.tile([G, C], F32)
            nc.vector.tensor_tensor(meansq[:], meanT[:], meanT[:], op=OP.mult)
            tmpv = stats.tile([G, C], F32)
            nc.vector.tensor_scalar_mul(tmpv[:], meansq[:], u[:])
            varT = stats.tile([G, C], F32)
            nc.vector.tensor_tensor(varT[:], m2T[:], tmpv[:], op=OP.subtract)
            # inv_std = exp(-0.5*ln(var+eps)): Ln+Exp live in one ACT table
            # set, so no table switch on the phase-1.5 critical path
            lnv = stats.tile([G, C], F32)
            nc.scalar.activation(lnv[:], varT[:], AF.Ln, bias=eps_t[:])
            invT = stats.tile([G, C], F32)
            nc.scalar.activation(invT[:], lnv[:], AF.Exp, scale=-0.5)
            Ff = stats.tile([G, C], F32)
            nc.vector.tensor_tensor(Ff[:], meanT[:], invT[:], op=OP.mult)
            E = stats.tile([G, C], F16)
            nc.scalar.copy(E[:], invT[:])
            F = stats.tile([G, C], F16)
            nc.scalar.copy(F[:], Ff[:])

        # ================= phase 2: apply =================
        NCH = 1024
        HC = 512
        with tc.tile_pool(name="agp_ps", bufs=2, space="PSUM") as agp_pool, \
             tc.tile_pool(name="a1_ps", bufs=2, space="PSUM") as a1_psum, \
             tc.tile_pool(name="a2_ps", bufs=2, space="PSUM") as a2_psum:
            for cc in range(N // NCH):
                n0 = cc * NCH
                agp = agp_pool.tile([G, NCH], F16)
                for r in range(NCH // TP):
                    t = cc * (NCH // TP) + r
                    nc.tensor.matmul(
                        agp[:, r * TP:(r + 1) * TP],
                        lhsT=a_t[:, 8 * t:8 * (t + 1)], rhs=ident[:],
                        is_transpose=True, start=(r == 0),
                        stop=(r == NCH // TP - 1), skip_group_check=True)
                ags = ag_sbp.tile([G, NCH], F16)
                if cc % 2 == 0:
                    nc.vector.tensor_copy(ags[:], agp[:])
                else:
                    nc.scalar.copy(ags[:], agp[:])
                os = opool.tile([C, NCH], F16)
                for h in range(2):
                    m0 = h * HC
                    a1p = a1_psum.tile([C, HC], F32)
                    a2p = a2_psum.tile([C, HC], F32)
                    nc.tensor.matmul(a1p[:], lhsT=E[:], rhs=ags[:, m0:m0 + HC],
                                     start=True, stop=True, skip_group_check=True)
                    nc.tensor.matmul(a2p[:], lhsT=F[:], rhs=ags[:, m0:m0 + HC],
                                     start=True, stop=True, skip_group_check=True)
                    a1s = a1s_pool.tile([C, HC], F16)
                    nc.scalar.copy(a1s[:], a1p[:])
                    tm = tm_pool.tile([C, HC], F16)
                    nc.vector.tensor_tensor(tm[:], xc[:, n0 + m0:n0 + m0 + HC],
                                            a1s[:], op=OP.mult)
                    nc.vector.tensor_tensor(os[:, m0:m0 + HC], tm[:], a2p[:],
                                            op=OP.subtract)
                nc.sync.dma_start(out_ext[:, n0:n0 + NCH], os[:])

    nc.compile()
    return nc


_CACHED_NC = None


def kernel(x, conv_w, conv_b):
    global _CACHED_NC
    x = np.asarray(x)
    conv_w = np.ascontiguousarray(conv_w, dtype=np.float32)
    conv_b = np.asarray(conv_b, dtype=np.float32)
    b, c, n = x.shape[0], x.shape[1], x.shape[2]
    if _CACHED_NC is None:
        _CACHED_NC = build_nc(N=n)
    nc = _CACHED_NC

    in_maps = [
        {
            "x": np.ascontiguousarray(x[i, :, :, 0], dtype=np.float32),
            "conv_w": conv_w,
            "conv_b": conv_b.reshape(1, -1),
        }
        for i in range(b)
    ]
    res = run_bass_kernel_spmd(nc, in_maps, core_ids=list(range(b)))
    out = np.stack([res.results[i]["out"] for i in range(b)])[..., None]
    return out.astype(np.float32)
